# revision 27
# baseline (speedup 1.0000x reference)
"""JANET 2-layer RNN kernel for 8 Trainium2 NeuronCores (v2).

T=512, B=64, D_IN=512, H=1024.  The recurrent scan is sequential, so it
is replicated on every core; the input projections are sharded and the
gathers are chunked so they hide under the scans.

  P0:  layer-0 input projections, sharded over T with an interleaved
       assignment (core r owns t = r + 8s) so that AllGather chunk c
       delivers the *contiguous* t-range [128c, 128(c+1)).  4 chunked
       AllGathers overlap with the S0 scan.
  S0:  layer-0 scan, replicated.  Per step: per-(gate, m-chunk) PSUM
       groups opened by an identity matmul that injects pf/pg (folds the
       +pf add into the PE), then k=0..3 matmuls (consuming half A of h)
       for all groups, then k=4..7 (half B), ordered so the EW chain of
       half A finishes before the next step's matmuls need it.  h is
       carried in bf16 (validated: mean rel err ~5e-3 vs 2e-2 budget).
  P1:  after each 128-step chunk of S0, the layer-1 input projection for
       that chunk is computed from Y0 (sharded over H: core r owns
       output rows [128r, 128(r+1))) and all-gathered; both hide under
       the continuing S0/S1 scans.
  S1:  layer-1 scan, replicated; writes Y1 bf16 (host casts to f32).

All per-core variation is in the input data (X slice, W1 slice), so the
SPMD program is identical on all cores.
"""
import sys, os
sys.path.insert(0, '/opt/trn_rl_repo')
import numpy as np

from concourse import bass, bacc, tile
from concourse.bass_utils import run_bass_kernel_spmd
import bass_rust

_add_dep = bass_rust.add_dep_helper

mybir = bass.mybir
dt = mybir.dt
AF = mybir.ActivationFunctionType
ALU = mybir.AluOpType

T, B, DIN, H = 512, 64, 512, 1024
BETA = 1.0
DEBUG_L0ONLY = False
DEBUG_PF0DUMP = False
NCORE = 8
JC = H // 128           # 8 h-chunks
KIN = DIN // 128        # 4 k-tiles for layer-0 input proj
PACK = JC * B           # 512 packed cols for h
HALF = PACK // 2        # 256
JH = JC // 2            # 4 chunks per half


def build_program(T_steps=T, debug_taps=False):
    NCH = max(1, T_steps // 128)   # collective chunks
    CT = T_steps // NCH            # steps per chunk
    SB = CT // NCORE               # local s-values per chunk per core
    NTBB = (T_steps // NCORE) * B  # per-core projection cols
    nc = bacc.Bacc("TRN2", target_bir_lowering=False, debug=False,
                   num_devices=NCORE)

    bf16 = dt.bfloat16
    f32 = dt.float32

    # ---- inputs (per-core data) ----
    XT_c = nc.declare_dram_parameter("XT_c", [KIN, 128, NTBB], bf16, isOutput=False)
    W0T = nc.declare_dram_parameter("W0T", [2, KIN, 128, JC, 128], bf16, isOutput=False)
    H0T = nc.declare_dram_parameter("H0T", [2, JC, 128, JC, 128], bf16, isOutput=False)
    W1T_c = nc.declare_dram_parameter("W1T_c", [2, JC, 128, 128], bf16, isOutput=False)
    H1T = nc.declare_dram_parameter("H1T", [2, JC, 128, JC, 128], bf16, isOutput=False)
    B0 = nc.declare_dram_parameter("B0", [2, JC, 128, 1], f32, isOutput=False)
    B1_c = nc.declare_dram_parameter("B1_c", [2, 128, 1], f32, isOutput=False)
    EYE = nc.declare_dram_parameter("EYE", [128, 128], bf16, isOutput=False)
    # NOTE: bf16 DRAM outputs read back as garbage through the PJRT path
    # (verified empirically: internal bf16 pipeline exact, every bf16
    # isOutput tensor corrupt).  Outputs must be f32.
    Y1 = nc.declare_dram_parameter("Y1", [JC, 128, T_steps, B], f32, isOutput=True)

    # ---- internal DRAM ----
    S = T_steps // NCORE           # s-values per core (t = r + 8s)
    # ladder of PF0 gather chunks (in s units): small first so the scan
    # can start early, large later for gather bandwidth
    if S >= 64:
        ladder = [(0, 8), (8, 8), (16, 16), (32, 16), (48, 16)]
        ladder = [(s0 * (S // 64), ns * (S // 64)) for s0, ns in ladder]
    else:
        ladder = [(0, S)]
    PF0loc = nc.dram_tensor("PF0loc", [S, 2, JC, 128, B], bf16)
    PF0g = [nc.dram_tensor(f"PF0g{i}", [NCORE, ns, 2, JC, 128, B], bf16,
                           addr_space="Shared")
            for i, (s0, ns) in enumerate(ladder)]
    Y0 = nc.dram_tensor("Y0", [JC, 128, T_steps, B], bf16)
    PF1loc = nc.dram_tensor("PF1loc", [NCH, 2, 128, CT * B], bf16)
    PF1 = nc.dram_tensor("PF1", [NCH, NCORE, 2, 128, CT * B], bf16,
                         addr_space="Shared")

    pf0loc_v = PF0loc.rearrange("s g m p b -> p s g m b")
    pf0_views = [t_.rearrange("n s g m p b -> n s g p m b") for t_ in PF0g]

    def pf0_src(t, g):
        s_glob, r = t // NCORE, t % NCORE
        for i, (s0, ns) in enumerate(ladder):
            if s0 <= s_glob < s0 + ns:
                return pf0_views[i][r, s_glob - s0, g]
        raise AssertionError(t)

    pf1_view = PF1.rearrange("c n g p (o b) -> c g p n o b", b=B)

    with tile.TileContext(nc) as tc:
        with tc.tile_pool(name="wts", bufs=1) as wpool:
            # both layers' recurrent weights resident for the whole kernel
            scanw = wpool.tile([128, 2 * 2 * JC * JC * 128], bf16)
            nc.sync.dma_start(scanw[:, :2 * JC * JC * 128],
                              H0T.rearrange("g k p m q -> p g k m q"))
            nc.sync.dma_start(scanw[:, 2 * JC * JC * 128:],
                              H1T.rearrange("g k p m q -> p g k m q"))
            eye_sb = wpool.tile([128, 128], bf16)
            nc.sync.dma_start(eye_sb[:], EYE.ap())
            w1_sb = wpool.tile([128, 2 * JC * 128], bf16)
            nc.sync.dma_start(w1_sb[:], W1T_c.rearrange("g k p q -> p g k q"))
            b1_sb = wpool.tile([128, 2], f32)
            nc.sync.dma_start(b1_sb[:], B1_c.rearrange("g p o -> p g o"))

            def wsl(layer, g, k, m):
                c = ((layer * 2 + g) * JC + k) * JC * 128 + m * 128
                return scanw[:, c:c + 128]

            ag0_insts = []
            ag1_insts = {}

            # ================= P0: layer-0 input projections ==========
            with tc.tile_pool(name="p0_x", bufs=1) as xpool, \
                 tc.tile_pool(name="p0_w", bufs=1) as p0wpool, \
                 tc.tile_pool(name="p0_ps", bufs=4, space="PSUM") as p0ps, \
                 tc.tile_pool(name="p0_out", bufs=4) as p0out, \
                 tc.tile_pool(name="p0_b", bufs=1) as p0b:
                w_sb = p0wpool.tile([128, 2 * KIN * JC * 128], bf16)
                nc.sync.dma_start(w_sb[:], W0T.rearrange("g k p m q -> p g k m q"))
                x_sb = xpool.tile([128, KIN * NTBB], bf16)
                nc.sync.dma_start(x_sb[:], XT_c.rearrange("k p n -> p k n"))
                b_sb = p0b.tile([128, 2 * JC], f32)
                nc.sync.dma_start(b_sb[:], B0.rearrange("g m p o -> p g m o"))

                NCHUNK = min(512, NTBB)
                SPC = NCHUNK // B          # s-values per col-chunk
                nxt = 0                    # next ladder chunk to gather
                for n in range(NTBB // NCHUNK):
                    for g in range(2):
                        for m in range(JC):
                            ps = p0ps.tile([128, NCHUNK], f32, tag="ps")
                            for k in range(KIN):
                                nc.tensor.matmul(
                                    ps[:],
                                    w_sb[:, ((g * KIN + k) * JC + m) * 128:
                                         ((g * KIN + k) * JC + m) * 128 + 128],
                                    x_sb[:, k * NTBB + n * NCHUNK:
                                         k * NTBB + (n + 1) * NCHUNK],
                                    start=(k == 0), stop=(k == KIN - 1))
                            ot = p0out.tile([128, NCHUNK], bf16, tag="ot")
                            nc.scalar.activation(ot[:], ps[:], AF.Identity,
                                                 bias=b_sb[:, g * JC + m:
                                                           g * JC + m + 1])
                            nc.sync.dma_start(
                                pf0loc_v[:, n * SPC:(n + 1) * SPC, g, m, :],
                                ot[:])
                    # gather every ladder chunk as soon as its s-range done
                    while nxt < len(ladder) and \
                            ladder[nxt][0] + ladder[nxt][1] <= (n + 1) * SPC:
                        s0, ns = ladder[nxt]
                        cc = nc.gpsimd.collective_compute(
                            "AllGather", ALU.bypass,
                            ins=[PF0loc.ap()[s0:s0 + ns].opt()],
                            outs=[PF0g[nxt].ap().opt()],
                            replica_groups=[list(range(NCORE))])
                        ag0_insts.append(cc.ins)
                        nxt += 1
                assert nxt == len(ladder)

            # ================= S0 (+ chunked P1 + gathers) ============
            def p1_chunk(c, pools):
                """Layer-1 input projection for steps [c*CT, (c+1)*CT),
                sharded over H (this core owns output rows of its rank,
                encoded in W1T_c), then all-gathered."""
                rhspool, pspool, outpool = pools
                NC2 = 256
                TCH = NC2 // B             # 4 timesteps per col-chunk
                GRP = min(8, CT * B // NC2)  # n-chunks per rhs load
                TG = GRP * TCH             # 32 timesteps per rhs load
                t0 = c * CT
                tags = ["psGA", "psGB", "psFA", "psFB"]
                for ng in range(CT * B // (NC2 * GRP)):
                    rhs = rhspool.tile([128, JC * NC2 * GRP], bf16, tag="p1rhs")
                    ta = t0 + ng * TG
                    for k in range(JC):
                        nc.sync.dma_start(
                            rhs[:, k * NC2 * GRP:(k + 1) * NC2 * GRP],
                            Y0.ap()[k, :, ta:ta + TG, :])
                    for ni in range(GRP):
                        n = ng * GRP + ni
                        for g in range(2):
                            ps = pspool.tile([128, NC2], f32,
                                             tag=tags[(2 * n + g) % 4],
                                             padded_shape=[128, 512])
                            for k in range(JC):
                                nc.tensor.matmul(
                                    ps[:],
                                    w1_sb[:, (g * JC + k) * 128:
                                          (g * JC + k) * 128 + 128],
                                    rhs[:, k * NC2 * GRP + ni * NC2:
                                        k * NC2 * GRP + (ni + 1) * NC2],
                                    start=(k == 0), stop=(k == JC - 1))
                            ot = outpool.tile([128, NC2], bf16, tag="p1out")
                            nc.scalar.activation(ot[:], ps[:], AF.Identity,
                                                 bias=b1_sb[:, g:g + 1])
                            nc.sync.dma_start(
                                PF1loc.ap()[c, g, :, n * NC2:(n + 1) * NC2],
                                ot[:])
                cc = nc.gpsimd.collective_compute(
                    "AllGather", ALU.bypass,
                    ins=[PF1loc.ap()[c].opt()], outs=[PF1.ap()[c].opt()],
                    replica_groups=[list(range(NCORE))])
                ag1_insts[c] = cc.ins

            def pf1_src(t, g):
                return pf1_view[t // CT, g, :, :, t % CT, :]

            def ag0_for(t):
                s_glob = t // NCORE
                for i, (s0, ns) in enumerate(ladder):
                    if s0 <= s_glob < s0 + ns:
                        return ag0_insts[i]
                return None

            def ag1_for(t):
                return ag1_insts.get(t // CT)

            if DEBUG_PF0DUMP:
                # dump gate-0 PF0 (as the scan would read it) into Y1:
                # Y1[j, p, t, b] = pf0(t)[p, (j, b)]
                with tc.tile_pool(name="dmp", bufs=4) as dpool:
                    yv = Y1.rearrange("j p t b -> p j t b")
                    for t in range(T_steps):
                        tl = dpool.tile([128, PACK], bf16, tag="d")
                        dd = nc.sync.dma_start(tl[:], pf0_src(t, 0))
                        ag = ag0_for(t)
                        if ag is not None:
                            _add_dep(dd.ins, ag, True, "dump after AG")
                        t32 = dpool.tile([128, PACK], f32, tag="d32")
                        nc.scalar.activation(t32[:], tl[:], AF.Identity)
                        nc.sync.dma_start(yv[:, :, t, :], t32[:])
            elif DEBUG_L0ONLY:
                scan_phase(nc, tc, 0, T_steps, CT, pf0_src, Y1, wsl, eye_sb,
                           insert_after_chunk=None, ag_for=ag0_for,
                           write_f32=True)
            else:
                scan_phase(nc, tc, 0, T_steps, CT, pf0_src, Y0, wsl, eye_sb,
                           insert_after_chunk=p1_chunk, ag_for=ag0_for)
                scan_phase(nc, tc, 1, T_steps, CT, pf1_src, Y1, wsl, eye_sb,
                           insert_after_chunk=None, ag_for=ag1_for)



    nc.compile()
    return nc


def scan_phase(nc, tc, layer, T_steps, CT, pf_src, yout, wsl, eye_sb,
               insert_after_chunk=None, ag_for=None, write_f32=None):
    if write_f32 is None:
        write_f32 = (layer == 1)
    bf16 = dt.bfloat16
    f32 = dt.float32
    yv = yout.rearrange("j p t b -> p j t b")

    with tc.tile_pool(name="s_pf", bufs=6) as pfpool, \
         tc.tile_pool(name="s_ps", bufs=2, space="PSUM") as pspool, \
         tc.tile_pool(name="s_h", bufs=3) as hpool, \
         tc.tile_pool(name="s_ew", bufs=2) as ewpool, \
         tc.tile_pool(name="s_p1r", bufs=2) as p1rhs, \
         tc.tile_pool(name="s_p1o", bufs=3) as p1out:
        hbA = hpool.tile([128, HALF], bf16, tag="hbA")
        hbB = hpool.tile([128, HALF], bf16, tag="hbB")
        nc.gpsimd.memset(hbA[:], 0.0)
        nc.gpsimd.memset(hbB[:], 0.0)

        PSPAD = [128, 512]  # one full PSUM bank per tile (no bank sharing)
        for t in range(T_steps):
            pf = pfpool.tile([128, PACK], bf16, tag="pf")
            pg = pfpool.tile([128, PACK], bf16, tag="pg")
            d0 = nc.sync.dma_start(pf[:], pf_src(t, 0))
            d1 = nc.sync.dma_start(pg[:], pf_src(t, 1))
            ag = ag_for(t) if ag_for is not None else None
            if ag is not None:
                # Tile does not order collective-output DRAM writes against
                # later DMA reads; add the edge explicitly.
                _add_dep(d0.ins, ag, True, "pf read after AllGather")
                _add_dep(d1.ins, ag, True, "pg read after AllGather")

            psGA = pspool.tile([128, HALF], f32, tag="psGA", padded_shape=PSPAD)
            psGB = pspool.tile([128, HALF], f32, tag="psGB", padded_shape=PSPAD)
            psFA = pspool.tile([128, HALF], f32, tag="psFA", padded_shape=PSPAD)
            psFB = pspool.tile([128, HALF], f32, tag="psFB", padded_shape=PSPAD)
            # groups: (psum, gate, m0, pf-source).  Order below controls
            # the PE stream: ids first (no h dependency), then k=0..3
            # (consume hbA), then k=4..7 (consume hbB); within the k=4..7
            # phase G-halves complete before F so tanh starts earliest.
            groups = [(psGA, 1, 0, pg), (psGB, 1, JH, pg),
                      (psFA, 0, 0, pf), (psFB, 0, JH, pf)]
            # ONE start=True matmul per PSUM tile: start clears the
            # has_written bits of the WHOLE bank, so it must be the single
            # first write covering every column of the tile.
            for ps, g, m0, src in groups:
                nc.tensor.matmul(ps[:], eye_sb[:],
                                 src[:, m0 * B:(m0 + JH) * B],
                                 start=True, stop=False, skip_group_check=True)
            for ps, g, m0, src in groups:
                for mi in range(JH):
                    m = m0 + mi
                    for k in range(JH):
                        nc.tensor.matmul(
                            ps[:, mi * B:(mi + 1) * B], wsl(layer, g, k, m),
                            hbA[:, k * B:(k + 1) * B],
                            start=False, stop=False)
            newh = {}
            for ps, g, m0, src in [groups[0], groups[2], groups[1], groups[3]]:
                for mi in range(JH):
                    m = m0 + mi
                    for k in range(JH):
                        nc.tensor.matmul(
                            ps[:, mi * B:(mi + 1) * B], wsl(layer, g, JH + k, m),
                            hbB[:, k * B:(k + 1) * B],
                            start=False, stop=(k == JH - 1))
                if g == 0:  # F group done; G group for this half already done
                    half = "A" if m0 == 0 else "B"
                    psG = psGA if m0 == 0 else psGB
                    hb_old = hbA if m0 == 0 else hbB
                    G = ewpool.tile([128, HALF], bf16, tag="G" + half)
                    nc.scalar.activation(G[:], psG[:], AF.Tanh)
                    F = ewpool.tile([128, HALF], bf16, tag="F" + half)
                    nc.scalar.activation(F[:], ps[:], AF.Sigmoid)
                    d = ewpool.tile([128, HALF], bf16, tag="d" + half)
                    nc.vector.tensor_sub(d[:], hb_old[:], G[:])
                    xm = ewpool.tile([128, HALF], bf16, tag="x" + half)
                    nc.vector.tensor_mul(xm[:], F[:], d[:])
                    nh = hpool.tile([128, HALF], bf16, tag="hb" + half)
                    nc.vector.tensor_add(nh[:], G[:], xm[:])
                    newh[half] = nh
                    jstart = 0 if m0 == 0 else JH
                    if write_f32:
                        # output must be f32 (bf16 DRAM outputs are broken
                        # through PJRT); parallel add off the h-chain
                        nh32 = ewpool.tile([128, HALF], f32, tag="y" + half)
                        nc.vector.tensor_add(nh32[:], G[:], xm[:])
                        nc.sync.dma_start(yv[:, jstart:jstart + JH, t, :], nh32[:])
                    else:
                        nc.sync.dma_start(yv[:, jstart:jstart + JH, t, :], nh[:])
            hbA, hbB = newh["A"], newh["B"]

            if insert_after_chunk is not None and (t + 1) % CT == 0:
                insert_after_chunk((t + 1) // CT - 1, (p1rhs, pspool, p1out))


# ----------------------------------------------------------------------
# host-side wrapper
# ----------------------------------------------------------------------
_cached = {}


def _get_program(T_steps):
    if T_steps not in _cached:
        _cached[T_steps] = build_program(T_steps)
    return _cached[T_steps]


def _bf16(a):
    import ml_dtypes
    return np.asarray(a, np.float32).astype(ml_dtypes.bfloat16)


def make_in_maps(inputs, T_steps=T):
    X = np.asarray(inputs["X"], np.float32)[:T_steps]

    def wT(w):  # [out, in] -> [in, out] reshaped [k,128,m,128]
        wt = np.ascontiguousarray(np.asarray(w, np.float32).T)
        ki, ko = wt.shape
        return wt.reshape(ki // 128, 128, ko // 128, 128)

    W0T = _bf16(np.stack([wT(inputs["ifW0"]), wT(inputs["igW0"])]))
    H0T = _bf16(np.stack([wT(inputs["hfW0"]), wT(inputs["hgW0"])]))
    W1T = np.stack([wT(inputs["ifW1"]), wT(inputs["igW1"])])  # [2,8,128,8,128]
    H1T = _bf16(np.stack([wT(inputs["hfW1"]), wT(inputs["hgW1"])]))
    B0 = np.stack([
        (inputs["ifB0"] + inputs["hfB0"] - BETA).astype(np.float32),
        (inputs["igB0"] + inputs["hgB0"]).astype(np.float32),
    ]).reshape(2, JC, 128, 1)
    B1 = np.stack([
        (inputs["ifB1"] + inputs["hfB1"] - BETA).astype(np.float32),
        (inputs["igB1"] + inputs["hgB1"]).astype(np.float32),
    ]).reshape(2, JC, 128, 1)
    EYE = _bf16(np.eye(128, dtype=np.float32))

    # X interleaved over T: core r owns t = r + NCORE*s
    XT = np.ascontiguousarray(X.transpose(2, 0, 1))  # [DIN, T, B]

    in_maps = []
    for r in range(NCORE):
        xs = XT[:, r::NCORE, :]  # [DIN, T/8, B]
        xs = np.ascontiguousarray(xs).reshape(KIN, 128, (T_steps // NCORE) * B)
        in_maps.append({
            "XT_c": _bf16(xs),
            "W0T": W0T,
            "H0T": H0T,
            "W1T_c": _bf16(W1T[:, :, :, r, :]),  # [2, 8, 128, 128]
            "H1T": H1T,
            "B0": B0,
            "B1_c": np.ascontiguousarray(B1[:, r]),
            "EYE": EYE,
        })
    return in_maps


def kernel(**inputs):
    T_steps = T
    nc = _get_program(T_steps)
    in_maps = make_in_maps(inputs, T_steps)
    res = run_bass_kernel_spmd(nc, in_maps, list(range(NCORE)))
    y = np.asarray(res.results[0]["Y1"], np.float32)  # [JC, 128, T, B]
    out = np.ascontiguousarray(y.transpose(2, 3, 0, 1).reshape(T_steps, B, H))
    return out
